# revision 3
# baseline (speedup 1.0000x reference)
"""Expert-parallel sparse MoE kernel for Trainium2 (8 NeuronCores).

Problem: gpt_oss-style top-2-of-8 MoE MLP over T=4096 tokens, H=1024,
I=1024. Sharding: expert-parallel — core c owns expert c's weights.

Host side (this file, numpy): compute router top-2 *indices* (the dispatch
decision), shard tokens to expert-owning cores (the "all-to-all dispatch" is
realized as host-side sharding since full inputs start on the host), and
scatter-add the per-expert outputs back into the full [T, H] output (the
"combine").

Device side (Bass/Tile, per core, SPMD): router logits + top-2 softmax
weights for its gathered tokens (router weight is column-permuted per core so
column 0 is always "own expert" — keeps the graph SPMD), fused gate/up
projection, gpt_oss GLU activation, down projection, scaling by the routing
weight. Big matmuls run in bf16 (PSUM accumulation in f32); routing weights
are computed from the same bf16 logits but selection comes from host f32
logits, so selection is never perturbed.
"""

import numpy as np
import ml_dtypes

import concourse.bass as bass
import concourse.tile as tile
from concourse import bacc, mybir
from concourse.bass_utils import run_bass_kernel_spmd

# Model dims (hardcoded per problem spec)
B, S, H, E, I, K = 2, 2048, 1024, 8, 1024, 2
ALPHA, LIMIT = 1.702, 7.0
T = B * S
P = 128
HB = H // P  # 8 h-chunks
IB = I // P  # 8 i-chunks
N_CORES = 8

BF16 = mybir.dt.bfloat16
F32 = mybir.dt.float32
NP_BF16 = ml_dtypes.bfloat16


def _ceil_to(x, m):
    return ((x + m - 1) // m) * m


def _chunks(total, step):
    out = []
    o = 0
    while o < total:
        w = min(step, total - o)
        out.append((o, w))
        o += w
    return out


def build_expert_kernel(C: int, has_bias: bool, reps: int = 1):
    """Build the per-core Bass graph. C = token capacity (multiple of 128).

    reps > 1 replicates the compute body (same inputs/outputs) inside one
    NEFF — used only for benchmarking (amortizes host dispatch overhead)."""
    assert C % P == 0
    CB = C // P

    nc = bacc.Bacc("TRN2", target_bir_lowering=False, debug=False,
                   num_devices=N_CORES)

    xg_ap = nc.dram_tensor("xg", [C, H], BF16, kind="ExternalInput").ap()
    wg_ap = nc.dram_tensor("wg", [H, I], BF16, kind="ExternalInput").ap()
    wu_ap = nc.dram_tensor("wu", [H, I], BF16, kind="ExternalInput").ap()
    wd_ap = nc.dram_tensor("wd", [I, H], BF16, kind="ExternalInput").ap()
    wr_ap = nc.dram_tensor("wr", [H, E], BF16, kind="ExternalInput").ap()
    if has_bias:
        bg_ap = nc.dram_tensor("bg", [P, IB], F32, kind="ExternalInput").ap()
        bu_ap = nc.dram_tensor("bu", [P, IB], F32, kind="ExternalInput").ap()
        bd_ap = nc.dram_tensor("bd", [P, H], F32, kind="ExternalInput").ap()
    y_ap = nc.dram_tensor("y", [C, H], F32, kind="ExternalOutput").ap()

    with tile.TileContext(nc) as tc:
        with (
            tc.tile_pool(name="weights", bufs=1) as wpool,
            tc.tile_pool(name="xgt", bufs=1) as xpool,
            tc.tile_pool(name="act", bufs=1) as apool,
            tc.tile_pool(name="router", bufs=2) as rpool,
            tc.tile_pool(name="elem", bufs=3) as epool,
            tc.tile_pool(name="yout", bufs=3) as ypool,
            tc.tile_pool(name="ps_r", bufs=2, space="PSUM") as ps_r,
            tc.tile_pool(name="ps_g", bufs=2, space="PSUM") as ps_g,
            tc.tile_pool(name="ps_u", bufs=2, space="PSUM") as ps_u,
            tc.tile_pool(name="ps_y", bufs=2, space="PSUM") as ps_y,
        ):
          for _rep in range(reps):
            # ---- load weights (SBUF resident) ----
            wg_sb = wpool.tile([P, HB, I], BF16)
            nc.sync.dma_start(wg_sb[:], wg_ap.rearrange("(ko p) i -> p ko i", p=P))
            wu_sb = wpool.tile([P, HB, I], BF16)
            nc.sync.dma_start(wu_sb[:], wu_ap.rearrange("(ko p) i -> p ko i", p=P))
            wd_sb = wpool.tile([P, IB, H], BF16)
            nc.sync.dma_start(wd_sb[:], wd_ap.rearrange("(ko p) i -> p ko i", p=P))
            wr_sb = wpool.tile([P, HB, E], BF16)
            nc.sync.dma_start(wr_sb[:], wr_ap.rearrange("(ko p) e -> p ko e", p=P))
            if has_bias:
                bg_sb = wpool.tile([P, IB], F32)
                nc.sync.dma_start(bg_sb[:], bg_ap[:, :])
                bu_sb = wpool.tile([P, IB], F32)
                nc.sync.dma_start(bu_sb[:], bu_ap[:, :])
                bd_sb = wpool.tile([P, H], F32)
                nc.sync.dma_start(bd_sb[:], bd_ap[:, :])

            # ---- load tokens transposed: xgT[p, ho, c] = xg[c, ho*128+p] ----
            xgT = xpool.tile([P, HB, C], BF16)
            for (n0, nw) in _chunks(C, 512):
                nc.sync.dma_start_transpose(
                    xgT[:, :, n0:n0 + nw], xg_ap[n0:n0 + nw, :])

            # ---- router: logits[c-part, e] for each 128-token block ----
            w_sb = rpool.tile([P, CB], F32, tag="wslot")  # routing weight per slot
            for cb in range(CB):
                ps_l = ps_r.tile([P, E], F32, space="PSUM")
                for hb in range(HB):
                    nc.tensor.matmul(
                        ps_l[:],
                        lhsT=xgT[:, hb, cb * P:(cb + 1) * P],
                        rhs=wr_sb[:, hb, :],
                        start=(hb == 0), stop=(hb == HB - 1),
                    )
                lg = rpool.tile([P, E], F32, tag="logits")
                nc.vector.tensor_copy(lg[:], ps_l[:])
                mx = rpool.tile([P, 8], F32, tag="mx")
                nc.vector.max(mx[:], lg[:])
                scratch = rpool.tile([P, 4], F32, tag="scratch")
                neg_m1 = scratch[:, 0:1]
                e2 = scratch[:, 1:2]
                rec = scratch[:, 2:3]
                numer = scratch[:, 3:4]
                nc.vector.tensor_scalar_mul(neg_m1, mx[:, 0:1], -1.0)
                # e2 = exp(m2 - m1)
                nc.scalar.activation(e2, mx[:, 1:2],
                                     mybir.ActivationFunctionType.Exp,
                                     bias=neg_m1)
                nc.vector.tensor_scalar_add(e2, e2, 1.0)
                nc.vector.reciprocal(rec, e2)
                # numer = exp(l_own - m1); own expert is column 0 by permutation
                nc.scalar.activation(numer, lg[:, 0:1],
                                     mybir.ActivationFunctionType.Exp,
                                     bias=neg_m1)
                nc.vector.tensor_mul(w_sb[:, cb:cb + 1], numer, rec)

            # ---- layer 1: gateT/upT [I-part, C] -> actT bf16 ----
            actT = apool.tile([P, IB, C], BF16)
            for m in range(IB):
                for (n0, nw) in _chunks(C, 512):
                    g_ps = ps_g.tile([P, 512], F32, space="PSUM", name="g_ps")[:, :nw]
                    u_ps = ps_u.tile([P, 512], F32, space="PSUM", name="u_ps")[:, :nw]
                    for hb in range(HB):
                        nc.tensor.matmul(
                            g_ps,
                            lhsT=wg_sb[:, hb, m * P:(m + 1) * P],
                            rhs=xgT[:, hb, n0:n0 + nw],
                            start=(hb == 0), stop=(hb == HB - 1))
                    for hb in range(HB):
                        nc.tensor.matmul(
                            u_ps,
                            lhsT=wu_sb[:, hb, m * P:(m + 1) * P],
                            rhs=xgT[:, hb, n0:n0 + nw],
                            start=(hb == 0), stop=(hb == HB - 1))
                    gc = epool.tile([P, 512], F32, tag="gc", name="gc")[:, :nw]
                    uc = epool.tile([P, 512], F32, tag="uc", name="uc")[:, :nw]
                    sg = epool.tile([P, 512], F32, tag="sg", name="sg")[:, :nw]
                    if has_bias:
                        nc.vector.tensor_add(
                            gc, g_ps, bg_sb[:, m:m + 1].to_broadcast([P, nw]))
                        nc.vector.tensor_scalar_min(gc, gc, LIMIT)
                        nc.vector.tensor_add(
                            uc, u_ps, bu_sb[:, m:m + 1].to_broadcast([P, nw]))
                        nc.vector.tensor_scalar(
                            uc, uc, LIMIT, -LIMIT,
                            mybir.AluOpType.min, mybir.AluOpType.max)
                    else:
                        nc.vector.tensor_scalar_min(gc, g_ps, LIMIT)
                        nc.vector.tensor_scalar(
                            uc, u_ps, LIMIT, -LIMIT,
                            mybir.AluOpType.min, mybir.AluOpType.max)
                    # sg = sigmoid(alpha * gate)
                    nc.scalar.activation(sg, gc,
                                         mybir.ActivationFunctionType.Sigmoid,
                                         scale=ALPHA)
                    # glu = gate * sg ; act = (up + 1) * glu
                    nc.vector.tensor_mul(gc, gc, sg)
                    nc.vector.tensor_scalar_add(uc, uc, 1.0)
                    nc.vector.tensor_mul(actT[:, m, n0:n0 + nw], uc, gc)

            # ---- layer 2: y[c-part, H] = actT.T @ wd, scaled by w ----
            for cb in range(CB):
                for (n0, nw) in _chunks(H, 512):
                    y_ps = ps_y.tile([P, 512], F32, space="PSUM", name="y_ps")[:, :nw]
                    for ib in range(IB):
                        nc.tensor.matmul(
                            y_ps,
                            lhsT=actT[:, ib, cb * P:(cb + 1) * P],
                            rhs=wd_sb[:, ib, n0:n0 + nw],
                            start=(ib == 0), stop=(ib == IB - 1))
                    y_sb = ypool.tile([P, 512], F32, tag="ysb", name="y_sb")[:, :nw]
                    if has_bias:
                        nc.vector.tensor_add(y_sb, y_ps, bd_sb[:, n0:n0 + nw])
                        nc.vector.tensor_mul(
                            y_sb, y_sb,
                            w_sb[:, cb:cb + 1].to_broadcast([P, nw]))
                    else:
                        nc.vector.tensor_mul(
                            y_sb, y_ps,
                            w_sb[:, cb:cb + 1].to_broadcast([P, nw]))
                    nc.sync.dma_start(
                        y_ap[cb * P:(cb + 1) * P, n0:n0 + nw], y_sb)

    nc.compile()
    return nc


_KERNEL_CACHE: dict = {}


def build_expert_kernel_replicated(C: int, has_bias: bool, reps: int):
    return build_expert_kernel(C, has_bias, reps)


def _get_kernel(C: int, has_bias: bool):
    key = (C, has_bias)
    if key not in _KERNEL_CACHE:
        _KERNEL_CACHE[key] = build_expert_kernel(C, has_bias)
    return _KERNEL_CACHE[key]


def _route(x, router_weight):
    """Host-side top-2 routing decision (indices only; weights computed on
    device). Mirrors jax.lax.top_k tie-breaking (first index wins)."""
    logits = x @ router_weight  # [T, E] f32
    # top-2 indices; argsort of -logits is stable so equals top_k on ties
    top2 = np.argsort(-logits, axis=1, kind="stable")[:, :K]
    return top2


def prepare_in_maps(hidden_states, router_weight, gate_up_proj,
                    gate_up_proj_bias, down_proj, down_proj_bias):
    x = np.ascontiguousarray(
        np.asarray(hidden_states, dtype=np.float32).reshape(T, H))
    rw = np.asarray(router_weight, dtype=np.float32)
    top2 = _route(x, rw)

    idx_lists = []
    for c in range(N_CORES):
        sel = np.nonzero((top2 == c).any(axis=1))[0]
        idx_lists.append(sel.astype(np.int64))
    max_load = max(len(s) for s in idx_lists)
    C = max(_ceil_to(max_load, P), 512)

    xbf = x.astype(NP_BF16)
    gup = np.asarray(gate_up_proj, dtype=np.float32)
    gub = np.asarray(gate_up_proj_bias, dtype=np.float32)
    dwn = np.asarray(down_proj, dtype=np.float32)
    dwb = np.asarray(down_proj_bias, dtype=np.float32)
    has_bias = bool(np.any(gub) or np.any(dwb))

    in_maps = []
    for c in range(N_CORES):
        idx = idx_lists[c]
        xg = np.zeros((C, H), dtype=NP_BF16)
        xg[:len(idx)] = xbf[idx]
        perm = [c] + [e for e in range(E) if e != c]
        m = {
            "xg": xg,
            "wg": np.ascontiguousarray(gup[c, :, 0::2]).astype(NP_BF16),
            "wu": np.ascontiguousarray(gup[c, :, 1::2]).astype(NP_BF16),
            "wd": np.ascontiguousarray(dwn[c]).astype(NP_BF16),
            "wr": np.ascontiguousarray(rw[:, perm]).astype(NP_BF16),
        }
        if has_bias:
            m["bg"] = np.ascontiguousarray(
                gub[c, 0::2].reshape(IB, P).T).astype(np.float32)
            m["bu"] = np.ascontiguousarray(
                gub[c, 1::2].reshape(IB, P).T).astype(np.float32)
            m["bd"] = np.broadcast_to(dwb[c], (P, H)).copy().astype(np.float32)
        in_maps.append(m)
    return in_maps, idx_lists, C, has_bias


def combine(results, idx_lists):
    out = np.zeros((T, H), dtype=np.float32)
    for c in range(N_CORES):
        idx = idx_lists[c]
        out[idx] += results[c]["y"][:len(idx)]
    return out.reshape(B, S, H)


def kernel(hidden_states, router_weight, gate_up_proj, gate_up_proj_bias,
           down_proj, down_proj_bias):
    in_maps, idx_lists, C, has_bias = prepare_in_maps(
        hidden_states, router_weight, gate_up_proj, gate_up_proj_bias,
        down_proj, down_proj_bias)
    nc = _get_kernel(C, has_bias)
    res = run_bass_kernel_spmd(nc, in_maps, core_ids=list(range(N_CORES)))
    return combine(res.results, idx_lists)
